# revision 48
# baseline (speedup 1.0000x reference)
"""Trainium2 Bass kernel v4 for nn_MultiHeadAttention_9912784519532.

Structure (per core: 2 heads of one batch):
  - projections qT/kT/v (PE), edge-folded k variants (DVE)
  - per 128-row q block J: wER row-block built on PE (h0/h1 packed via
    tile_position row split), copied PSUM->SBUF (DVE), then the diagonal
    window extracted by an SBUF->SBUF shear DMA (per-partition shift)
  - flash over (qc of 512 q, kt of 128 keys): S^T h0/h1 packed into one
    2-bank zt [128,1024]; window bias accumulated via transpose-as-matmul
    (lhsT=window, rhs=bf16 identity); ONE exp per kt (ACT); attn@v lagged
    one iteration so PE never head-blocks on ACT
  - wER builds for qc+1 and normalization/output-projection of qc-1 are
    emitted as filler inside the kt loop to keep PE dense (HAM stays warm)
  - normalization: DVE reciprocal on the denominator row in place, then a
    stride-0 broadcast DMA replicates it across 64 partitions (no PE)
  v4 changes vs v3:
  - warmup matmuls read a gpsimd-memset tile (no DMA dependency) so PE
    warms at ~7us instead of ~12us
  - input DMAs fanned out across engine queues with host-side contiguous
    layouts (no in-DMA rearranges)
  - sel-matmul reciprocal broadcast replaced by broadcast DMA (saves ~10us
    of PE wall)
  - final-qc normalization reads oth PSUM directly (shorter tail)
"""

import numpy as np

HEADS = 8
D = 64
N = 2048
DIM = 512
WER = 1280
P = 128
QW = 512
NT = N // P      # 16 key tiles
NQC = N // QW    # 4 q chunks
NJ = QW // P     # 4 q blocks per chunk

_cached = {}


def _build_program():
    import concourse.bass as bass
    import concourse.mybir as mybir
    import concourse.tile as tile
    from concourse import bacc

    f32 = mybir.dt.float32
    bf16 = mybir.dt.bfloat16
    AP = bass.AP

    nc = bacc.Bacc(
        "TRN2",
        target_bir_lowering=False,
        debug=False,
        enable_asserts=False,
        num_devices=8,
    )

    xT_d = nc.dram_tensor("xT4", [4, P, 4, QW], bf16, kind="ExternalInput")
    wq_d = nc.dram_tensor("wq2", [P, 4, P], bf16, kind="ExternalInput")
    wk_d = nc.dram_tensor("wk2", [P, 4, P], bf16, kind="ExternalInput")
    wv_d = nc.dram_tensor("wv2", [P, 4, P], bf16, kind="ExternalInput")
    wo_d = nc.dram_tensor("wo2", [64, 2, DIM], bf16, kind="ExternalInput")
    relx_d = nc.dram_tensor("relx2", [P, WER], bf16, kind="ExternalInput")
    edge_d = nc.dram_tensor("edge2", [P, 2], f32, kind="ExternalInput")
    ident_d = nc.dram_tensor("identw", [P, P], bf16, kind="ExternalInput")
    sel_d = nc.dram_tensor("sel2", [2, 2 * 64], bf16, kind="ExternalInput")
    y_d = nc.dram_tensor("y", [N, DIM], bf16, kind="ExternalOutput")

    SCALE = float(D) ** -0.5

    with tile.TileContext(nc) as tc:
        import contextlib

        ctx = contextlib.ExitStack()
        with ctx:
            const = ctx.enter_context(tc.tile_pool(name="const", bufs=1))
            big = ctx.enter_context(tc.tile_pool(name="big", bufs=1))
            cpool = ctx.enter_context(tc.tile_pool(name="wstage", bufs=6))
            ypool = ctx.enter_context(tc.tile_pool(name="ycopy", bufs=4))
            npool = ctx.enter_context(tc.tile_pool(name="norm", bufs=2))
            spool = ctx.enter_context(tc.tile_pool(name="ost", bufs=3))
            apool = ctx.enter_context(tc.tile_pool(name="attn", bufs=4))
            wpool = ctx.enter_context(tc.tile_pool(name="win", bufs=20))
            # PSUM: ppool 2 + zpool 4 + opool 2 = 8 banks
            ppool = ctx.enter_context(tc.tile_pool(name="ps", bufs=2, space="PSUM"))
            zpool = ctx.enter_context(tc.tile_pool(name="zs", bufs=2, space="PSUM"))
            opool = ctx.enter_context(tc.tile_pool(name="ot", bufs=2, space="PSUM"))

            # ---- warmup tile: no DMA dependency, so the PE HAM warms up
            # while the input DMAs are still in flight ----
            warm_w = const.tile([P, QW], bf16)
            nc.gpsimd.memset(warm_w[:], 0.001)

            # ---- input DMAs fanned out across engine SWDGE queues ----
            # scalar: wq, wk (gate the first projections)
            wq_sb = const.tile([P, 4, P], bf16)
            nc.scalar.dma_start(wq_sb[:], wq_d.ap())
            wk_sb = const.tile([P, 4, P], bf16)
            nc.scalar.dma_start(wk_sb[:], wk_d.ap())
            # sync: the big x tiles (xtn0 in per-cc chunks so the first
            # projection matmul can start as soon as chunk 0 lands)
            xtn = [
                big.tile([P, 4, QW], bf16, name=f"xt{nch}") for nch in range(4)
            ]
            for cc in range(4):
                nc.sync.dma_start(xtn[0][:, cc, :], xT_d.ap()[0][:, cc, :])
            nc.sync.dma_start(xtn[1][:], xT_d.ap()[1])
            nc.sync.dma_start(xtn[2][:], xT_d.ap()[2])
            nc.sync.dma_start(xtn[3][:], xT_d.ap()[3])
            # scalar (after wq/wk): relx, wv
            relx_sb = const.tile([P, WER], bf16)
            nc.scalar.dma_start(relx_sb[:], relx_d.ap())
            wv_sb = const.tile([P, 4, P], bf16)
            nc.scalar.dma_start(wv_sb[:], wv_d.ap())
            # gpsimd (after the memset): ident, edge, wo
            identw = const.tile([P, P], bf16)
            nc.gpsimd.dma_start(identw[:], ident_d.ap())
            ident_sb = identw[:, 0:P]
            edge_sb = const.tile([P, 2], f32)
            nc.gpsimd.dma_start(edge_sb[:], edge_d.ap())
            wo_sb = const.tile([64, 2, DIM], bf16)
            nc.gpsimd.dma_start(wo_sb[:], wo_d.ap())

            # ---- PE warmup: dummy matmuls so HAM reaches K=8/8 before
            # the real work arrives (only depends on the gpsimd memset) ----
            wz = zpool.tile([P, 2, QW], f32, name="warm", tag="zs")
            for i in range(9):
                nc.tensor.matmul(
                    wz[:, 0, :], warm_w[:, 0:P], warm_w[:],
                    start=True, stop=True, skip_group_check=True,
                )

            # ---- wER build blocks (emitted inline or as filler) ----
            windows = {}   # (h, J) -> (window tile, r0)
            staging = {}   # J -> [wtile_h0, wtile_h1]
            qtn = [big.tile([P, QW], bf16, name=f"qt{i}") for i in range(4)]

            def emit_build_chunk(J, ci, on_act=False, use_zpool=False):
                qb = J * P
                c0, cw = ((0, QW), (QW, QW), (2 * QW, WER - 2 * QW))[ci]
                if ci == 0:
                    staging[J] = [
                        cpool.tile([P, WER], bf16, name=f"wst{h}", tag="wst")
                        for h in range(2)
                    ]
                if use_zpool:
                    pair = zpool.tile([P, 2, QW], f32, name="wpz", tag="zs")
                    pts = [pair[:, 0, :], pair[:, 1, :]]
                else:
                    pts = [
                        ppool.tile([P, QW], f32, name=f"wps{h}", tag="ps")[:]
                        for h in range(2)
                    ]
                for h in range(2):
                    hs = slice(h * 64, h * 64 + 64)
                    nc.tensor.matmul(
                        pts[h][:, :cw],
                        qtn[J // 4][hs, (J % 4) * P : (J % 4 + 1) * P],
                        relx_sb[hs, c0 : c0 + cw],
                        start=True,
                        stop=True,
                    )
                for h in range(2):
                    if on_act and h == 0:
                        nc.scalar.copy(
                            staging[J][h][:, c0 : c0 + cw], pts[h][:, :cw]
                        )
                    else:
                        nc.vector.tensor_copy(
                            staging[J][h][:, c0 : c0 + cw], pts[h][:, :cw]
                        )

            def emit_shear(J):
                qb = J * P
                r0 = max(0, qb - 512)
                r1 = min(N, qb + 640)
                rw = r1 - r0
                for h in range(2):
                    wt = wpool.tile([P, 1152], bf16, name=f"win{h}", tag="win")
                    src = AP(
                        tensor=staging[J][h][:].tensor,
                        offset=640 + r0 - qb,
                        ap=[[WER - 1, P], [1, rw]],
                    )
                    nc.gpsimd.dma_start(wt[:, :rw], src)
                    windows[(h, J)] = (wt, r0)
                del staging[J]

            def build_filler_js(js, on_act=False, use_zpool=False):
                out = []
                for J in js:
                    for ci in range(3):
                        out.append(
                            lambda J=J, ci=ci: emit_build_chunk(
                                J, ci, on_act, use_zpool
                            )
                        )
                    out.append(lambda J=J: emit_shear(J))
                return out

            bitems0 = build_filler_js([3, 2, 1, 0], on_act=True)
            ebi = 0

            # ---- projections: qT2/kT2 ----
            kt2 = big.tile([P, N], bf16)
            for nch in range(4):
                for which, wsb in (("q", wq_sb), ("k", wk_sb)):
                    pt = ppool.tile([P, QW], f32, name="proj", tag="ps")
                    for cc in range(4):
                        nc.tensor.matmul(
                            pt[:],
                            wsb[:, cc, :],
                            xtn[nch][:, cc, :],
                            start=(cc == 0),
                            stop=(cc == 3),
                        )
                    if which == "q":
                        nc.vector.tensor_copy(qtn[nch][:], pt[:])
                    else:
                        nc.vector.tensor_copy(
                            kt2[:, nch * QW : (nch + 1) * QW], pt[:]
                        )
                    if nch >= 1:
                        for _ in range(2):
                            if ebi < 8:
                                bitems0[ebi]()
                                ebi += 1

            ktp = big.tile([P, N], bf16)
            ktf = big.tile([P, N], bf16)
            nc.vector.tensor_scalar_add(ktp[:], kt2[:], edge_sb[:, 0:1])
            nc.vector.tensor_scalar_add(ktf[:], kt2[:], edge_sb[:, 1:2])

            # ---- v2 = [v | 1]: vproj for kt>=2 happens as filler inside
            # qc0's flash (keeps PE dense); lead phase = J0-3 builds only ----
            v2 = big.tile([P, 2, NT, 65], bf16)

            def emit_vproj(kt):
                pt = ppool.tile([P, QW], f32, name="vproj", tag="ps")
                for cc in range(4):
                    nc.tensor.matmul(
                        pt[:, :P],
                        xtn[kt // 4][:, cc, (kt % 4) * P : (kt % 4 + 1) * P],
                        wv_sb[:, cc, :],
                        start=(cc == 0),
                        stop=(cc == 3),
                    )
                nc.vector.tensor_copy(
                    v2[:, 0:2, kt, 0:64],
                    AP(
                        tensor=pt[:].tensor,
                        offset=0,
                        ap=[[QW, P], [64, 2], [1, 64]],
                    ),
                )

            nc.vector.memset(v2[:], 1.0)
            emit_vproj(NT - 1)
            emit_vproj(NT - 2)

            # ---- normalization + output projection (for a finished qc) ----
            otn = big.tile([64, 2, QW], bf16)
            sel_sb = const.tile([2, 2, 64], bf16)
            nc.gpsimd.dma_start(sel_sb[:], sel_d.ap())

            def build_filler(qc, on_act=False, use_zpool=False):
                return build_filler_js(
                    range(qc * NJ, (qc + 1) * NJ), on_act, use_zpool
                )

            def emit_norm_head(qc, ost, rrow):
                # den rows -> [64,16] -> reciprocal -> bf16 row form
                # (DVE reciprocal costs ~8 cyc/free-elem, so shrink the
                # free dim via DMA reshape before it)
                den16 = npool.tile([64, 16], f32, name="den16", tag="nrm")
                for h in range(2):
                    dst = AP(
                        tensor=den16[:].tensor,
                        offset=h * 8,
                        ap=[[16, 64], [1, 8]],
                    )
                    nc.sync.dma_start(dst, ost[h][64:65, :])
                rden16 = npool.tile([64, 16], f32, name="rden16", tag="nrm2")
                nc.vector.reciprocal(rden16[:], den16[:])
                rden16b = npool.tile([64, 16], bf16, name="rden16b", tag="nrm3")
                nc.vector.tensor_copy(rden16b[:], rden16[:])
                for h in range(2):
                    src = AP(
                        tensor=rden16b[:].tensor,
                        offset=h * 8,
                        ap=[[16, 64], [1, 8]],
                    )
                    dst = AP(
                        tensor=rrow[:].tensor,
                        offset=h * QW,
                        ap=[[QW, 1], [1, QW]],
                    )
                    nc.sync.dma_start(dst, src)

            def norm_filler(qc, ost, rrow, final=False):
                # emitted during qc+1 (or inline at the very end for the
                # final qc): broadcast recip + normalize + project
                out = []

                def emit_rcb_mul(h):
                    rcb = ppool.tile([64, QW], f32, name="rcb", tag="ps")
                    nc.tensor.matmul(
                        rcb[:], sel_sb[:, h, :], rrow[:], start=True, stop=True
                    )
                    nc.vector.tensor_mul(
                        otn[:, h, :], ost[h][0:64, :], rcb[:]
                    )

                def emit_yproj(nt):
                    pt = ppool.tile([P, QW], f32, name="yproj", tag="ps")
                    for h in range(2):
                        nc.tensor.matmul(
                            pt[:],
                            otn[:, h, nt * P : (nt + 1) * P],
                            wo_sb[:, h, :],
                            start=(h == 0),
                            stop=(h == 1),
                        )
                    yt = ypool.tile([P, QW], bf16, name="y_sb")
                    if final:
                        nc.scalar.copy(yt[:], pt[:])
                    else:
                        nc.vector.tensor_copy(yt[:], pt[:])
                    nc.sync.dma_start(
                        y_d.ap()[(qc * NJ + nt) * P : (qc * NJ + nt + 1) * P, :],
                        yt[:],
                    )

                out.append(lambda: emit_rcb_mul(0))
                out.append(lambda: emit_rcb_mul(1))
                for nt in range(NJ):
                    out.append(lambda nt=nt: emit_yproj(nt))
                return out

            # ---- flash loop ----
            filler = []
            pending_norm = None
            deferred_norm = []
            for qc in range(NQC):
                if qc == 0:
                    # qc0 runs kt DESCENDING: kt 15..8 need no J0-3 windows,
                    # so those builds become fillers (2 builds + 1 vproj per
                    # iteration; J3 first since descending kt hits it first)
                    bitems = bitems0[ebi:]
                    bitems += build_filler_js([4, 5, 6, 7])
                    vitems = [
                        lambda kt=k: emit_vproj(kt)
                        for k in range(NT - 3, -1, -1)
                    ]
                    bi = vi = 0
                    while bi < len(bitems) or vi < len(vitems):
                        for _ in range(2):
                            if bi < len(bitems):
                                filler.append(bitems[bi])
                                bi += 1
                        if vi < len(vitems):
                            filler.append(vitems[vi])
                            vi += 1
                else:
                    bitems = (
                        build_filler(qc + 1) if qc + 1 < NQC else []
                    )
                    if qc == NQC - 1:
                        # defer qc2's norm/yproj past the final attn@v:
                        # it is ready PE work that fills the final-qc
                        # norm chain's DMA latency instead of draining
                        # in qc3's first iterations
                        deferred_norm = pending_norm or []
                        nitems = []
                    else:
                        nitems = pending_norm or []
                    pending_norm = None
                    # a few builds first (norm DMA chain needs ~4us of
                    # latency cover), then interleave norm work among the
                    # remaining builds so the qc boundary never drains a
                    # long dependent chain with PE idle
                    filler += bitems[:4]
                    rest = bitems[4:]
                    i = j = 0
                    while i < len(nitems) or j < len(rest):
                        if i < len(nitems):
                            filler.append(nitems[i])
                            i += 1
                        if j < len(rest):
                            filler.append(rest[j])
                            j += 1


                oth = [
                    opool.tile([65, QW], f32, name=f"oth{h}", tag="ot")
                    for h in range(2)
                ]
                kts = list(range(NT - 1, -1, -1)) if qc == 0 else list(range(NT))
                prev = None  # (at tile, kt) pending attn@v
                for kt in kts:
                    kb = kt * P
                    cls = []
                    for j in range(NJ):
                        dlt = (qc * NJ + j) * P - kb
                        cls.append(
                            "p" if dlt >= 640 else ("f" if dlt <= -640 else "w")
                        )
                    runs = []
                    for j in range(NJ):
                        if runs and runs[-1][2] == cls[j]:
                            runs[-1][1] += P
                        else:
                            runs.append([j * P, P, cls[j]])
                    zt = zpool.tile([P, 2, QW], f32, name="zt", tag="zs")
                    for ri, (s, wd, c) in enumerate(runs):
                        kvar = {"p": ktp, "f": ktf, "w": kt2}[c]
                        for h in range(2):
                            hs = slice(h * 64, h * 64 + 64)
                            nc.tensor.matmul(
                                zt[:, h, s : s + wd],
                                kvar[hs, kb : kb + P],
                                qtn[qc][hs, s : s + wd],
                                start=(ri == 0),
                                stop=False,
                                skip_group_check=True,
                            )
                    for h in range(2):
                        for j in range(NJ):
                            if cls[j] != "w":
                                continue
                            wt, r0 = windows[(h, qc * NJ + j)]
                            nc.tensor.matmul(
                                zt[:, h, j * P : (j + 1) * P],
                                wt[:, kb - r0 : kb - r0 + P],
                                ident_sb,
                                start=False,
                                stop=False,
                                skip_group_check=True,
                            )
                    at = apool.tile([P, 2, QW], bf16, name="attnT")
                    nc.scalar.activation(
                        at[:], zt[:], mybir.ActivationFunctionType.Exp,
                        scale=SCALE,
                    )
                    if prev is not None:
                        pat, pkt = prev
                        for h in range(2):
                            nc.tensor.matmul(
                                oth[h][:],
                                v2[:, h, pkt, :],
                                pat[:, h, :],
                                start=(pkt == kts[0]),
                                stop=False,
                                skip_group_check=True,
                            )
                    prev = (at, kt)
                    for _ in range(3 if qc == 0 else 2):
                        if filler:
                            filler.pop(0)()
                # last attn@v
                pat, pkt = prev
                for h in range(2):
                    nc.tensor.matmul(
                        oth[h][:],
                        v2[:, h, pkt, :],
                        pat[:, h, :],
                        start=False,
                        stop=True,
                        skip_group_check=True,
                    )
                while filler:
                    filler.pop(0)()
                # stage out + denominators for this qc
                fin = qc == NQC - 1
                ost = []
                for h in range(2):
                    o = spool.tile([65, QW], f32, name=f"ost{h}", tag="ost")
                    if fin and h == 1:
                        nc.vector.tensor_copy(o[:], oth[h][:])
                    else:
                        nc.scalar.copy(o[:], oth[h][:])
                    ost.append(o)
                rrow = npool.tile([2, QW], bf16, name="rrow", tag="nrmb")
                emit_norm_head(qc, ost, rrow)
                pending_norm = norm_filler(qc, ost, rrow, final=fin)

            for f in deferred_norm + pending_norm:
                f()

    nc.compile()
    return nc


def _host_prep(x, Wq, Wkv, Wo, rel_emb):
    import ml_dtypes

    bf = ml_dtypes.bfloat16
    ident = np.eye(P, dtype=np.float32).astype(bf)
    sel2 = np.zeros((2, 2, 64), np.float32)
    for i in range(2):
        sel2[i, i, :] = 1.0
    sel2 = np.ascontiguousarray(sel2.reshape(2, 2 * 64)).astype(bf)
    relX = rel_emb[np.clip(1152 - np.arange(WER), 0, 1024)].T
    relx2 = np.ascontiguousarray(np.concatenate([relX, relX], axis=0).astype(bf))
    edge = np.stack([rel_emb[1024], rel_emb[0]], axis=1)
    edge2 = np.ascontiguousarray(
        np.concatenate([edge, edge], axis=0).astype(np.float32)
    )
    Wkv_r = Wkv.reshape(DIM, 2, HEADS, D)

    def wlayout(w):
        # [512, 128] -> [128 part, 4 chunk, 128] contiguous
        return np.ascontiguousarray(
            w.reshape(4, P, P).transpose(1, 0, 2).astype(bf)
        )

    in_maps = []
    for core in range(8):
        b = core // 4
        h0 = 2 * (core % 4)
        in_maps.append(
            {
                "xT4": np.ascontiguousarray(
                    x[b].T.astype(bf).reshape(4, P, 4, QW).transpose(2, 1, 0, 3)
                ),
                "wq2": wlayout(Wq[:, h0 * D : (h0 + 2) * D]),
                "wk2": wlayout(Wkv_r[:, 0, h0 : h0 + 2].reshape(DIM, 2 * D)),
                "wv2": wlayout(Wkv_r[:, 1, h0 : h0 + 2].reshape(DIM, 2 * D)),
                "wo2": np.ascontiguousarray(
                    Wo[h0 * D : (h0 + 2) * D, :]
                    .reshape(2, D, DIM)
                    .transpose(1, 0, 2)
                    .astype(bf)
                ),
                "relx2": relx2,
                "edge2": edge2,
                "identw": ident,
                "sel2": sel2,
            }
        )
    return in_maps


def kernel(x, Wq, Wkv, Wo, bo, rel_emb, _want_trace=False):
    from concourse.bass_utils import run_bass_kernel_spmd

    x = np.asarray(x)
    if "nc" not in _cached:
        _cached["nc"] = _build_program()
    nc = _cached["nc"]
    in_maps = _host_prep(x, np.asarray(Wq), np.asarray(Wkv), np.asarray(Wo),
                         np.asarray(rel_emb))
    res = run_bass_kernel_spmd(
        nc, in_maps, core_ids=list(range(8)), trace=_want_trace
    )
    _cached["last_result"] = res
    y = np.zeros((2, N, DIM), np.float32)
    for core in range(8):
        y[core // 4] += res.results[core]["y"].astype(np.float32)
    y += np.asarray(bo).astype(np.float32)[None, None, :]
    return y


# revision 50
# speedup vs baseline: 1.0044x; 1.0044x over previous
"""Trainium2 Bass kernel v4 for nn_MultiHeadAttention_9912784519532.

Structure (per core: 2 heads of one batch):
  - projections qT/kT/v (PE), edge-folded k variants (DVE)
  - per 128-row q block J: wER row-block built on PE (h0/h1 packed via
    tile_position row split), copied PSUM->SBUF (DVE), then the diagonal
    window extracted by an SBUF->SBUF shear DMA (per-partition shift)
  - flash over (qc of 512 q, kt of 128 keys): S^T h0/h1 packed into one
    2-bank zt [128,1024]; window bias accumulated via transpose-as-matmul
    (lhsT=window, rhs=bf16 identity); ONE exp per kt (ACT); attn@v lagged
    one iteration so PE never head-blocks on ACT
  - wER builds for qc+1 and normalization/output-projection of qc-1 are
    emitted as filler inside the kt loop to keep PE dense (HAM stays warm)
  - normalization: DVE reciprocal on the denominator row in place, then a
    stride-0 broadcast DMA replicates it across 64 partitions (no PE)
  v4 changes vs v3:
  - warmup matmuls read a gpsimd-memset tile (no DMA dependency) so PE
    warms at ~7us instead of ~12us
  - input DMAs fanned out across engine queues with host-side contiguous
    layouts (no in-DMA rearranges)
  - sel-matmul reciprocal broadcast replaced by broadcast DMA (saves ~10us
    of PE wall)
  - final-qc normalization reads oth PSUM directly (shorter tail)
"""

import numpy as np

HEADS = 8
D = 64
N = 2048
DIM = 512
WER = 1280
P = 128
QW = 512
NT = N // P      # 16 key tiles
NQC = N // QW    # 4 q chunks
NJ = QW // P     # 4 q blocks per chunk

_cached = {}


def _build_program():
    import concourse.bass as bass
    import concourse.mybir as mybir
    import concourse.tile as tile
    from concourse import bacc

    f32 = mybir.dt.float32
    bf16 = mybir.dt.bfloat16
    AP = bass.AP

    nc = bacc.Bacc(
        "TRN2",
        target_bir_lowering=False,
        debug=False,
        enable_asserts=False,
        num_devices=8,
    )

    xT_d = nc.dram_tensor("xT4", [4, P, 4, QW], bf16, kind="ExternalInput")
    wq_d = nc.dram_tensor("wq2", [P, 4, P], bf16, kind="ExternalInput")
    wk_d = nc.dram_tensor("wk2", [P, 4, P], bf16, kind="ExternalInput")
    wv_d = nc.dram_tensor("wv2", [P, 4, P], bf16, kind="ExternalInput")
    wo_d = nc.dram_tensor("wo2", [64, 2, DIM], bf16, kind="ExternalInput")
    relx_d = nc.dram_tensor("relx2", [P, WER], bf16, kind="ExternalInput")
    edge_d = nc.dram_tensor("edge2", [P, 2], f32, kind="ExternalInput")
    ident_d = nc.dram_tensor("identw", [P, P], bf16, kind="ExternalInput")
    sel_d = nc.dram_tensor("sel2", [2, 2 * 64], bf16, kind="ExternalInput")
    y_d = nc.dram_tensor("y", [N, DIM], bf16, kind="ExternalOutput")

    SCALE = float(D) ** -0.5

    with tile.TileContext(nc) as tc:
        import contextlib

        ctx = contextlib.ExitStack()
        with ctx:
            const = ctx.enter_context(tc.tile_pool(name="const", bufs=1))
            big = ctx.enter_context(tc.tile_pool(name="big", bufs=1))
            cpool = ctx.enter_context(tc.tile_pool(name="wstage", bufs=8))
            ypool = ctx.enter_context(tc.tile_pool(name="ycopy", bufs=4))
            npool = ctx.enter_context(tc.tile_pool(name="norm", bufs=2))
            spool = ctx.enter_context(tc.tile_pool(name="ost", bufs=3))
            apool = ctx.enter_context(tc.tile_pool(name="attn", bufs=4))
            wpool = ctx.enter_context(tc.tile_pool(name="win", bufs=24))
            # PSUM: ppool 2 + zpool 4 + opool 2 = 8 banks
            ppool = ctx.enter_context(tc.tile_pool(name="ps", bufs=2, space="PSUM"))
            zpool = ctx.enter_context(tc.tile_pool(name="zs", bufs=2, space="PSUM"))
            opool = ctx.enter_context(tc.tile_pool(name="ot", bufs=2, space="PSUM"))

            # ---- warmup tile: no DMA dependency, so the PE HAM warms up
            # while the input DMAs are still in flight ----
            warm_w = const.tile([P, QW], bf16)
            nc.gpsimd.memset(warm_w[:], 0.001)

            # ---- input DMAs fanned out across engine SWDGE queues ----
            # scalar: wq, wk (gate the first projections)
            wq_sb = const.tile([P, 4, P], bf16)
            nc.scalar.dma_start(wq_sb[:], wq_d.ap())
            wk_sb = const.tile([P, 4, P], bf16)
            nc.scalar.dma_start(wk_sb[:], wk_d.ap())
            # sync: the big x tiles (xtn0 in per-cc chunks so the first
            # projection matmul can start as soon as chunk 0 lands)
            xtn = [
                big.tile([P, 4, QW], bf16, name=f"xt{nch}") for nch in range(4)
            ]
            for cc in range(4):
                nc.sync.dma_start(xtn[0][:, cc, :], xT_d.ap()[0][:, cc, :])
            nc.sync.dma_start(xtn[1][:], xT_d.ap()[1])
            nc.sync.dma_start(xtn[2][:], xT_d.ap()[2])
            nc.sync.dma_start(xtn[3][:], xT_d.ap()[3])
            # scalar (after wq/wk): relx, wv
            relx_sb = const.tile([P, WER], bf16)
            nc.scalar.dma_start(relx_sb[:], relx_d.ap())
            wv_sb = const.tile([P, 4, P], bf16)
            nc.scalar.dma_start(wv_sb[:], wv_d.ap())
            # gpsimd (after the memset): ident, edge, wo
            identw = const.tile([P, P], bf16)
            nc.gpsimd.dma_start(identw[:], ident_d.ap())
            ident_sb = identw[:, 0:P]
            edge_sb = const.tile([P, 2], f32)
            nc.gpsimd.dma_start(edge_sb[:], edge_d.ap())
            wo_sb = const.tile([64, 2, DIM], bf16)
            nc.gpsimd.dma_start(wo_sb[:], wo_d.ap())

            # ---- PE warmup: dummy matmuls so HAM reaches K=8/8 before
            # the real work arrives (only depends on the gpsimd memset) ----
            wz = zpool.tile([P, 2, QW], f32, name="warm", tag="zs")
            for i in range(9):
                nc.tensor.matmul(
                    wz[:, 0, :], warm_w[:, 0:P], warm_w[:],
                    start=True, stop=True, skip_group_check=True,
                )

            # ---- wER build blocks (emitted inline or as filler) ----
            windows = {}   # (h, J) -> (window tile, r0)
            staging = {}   # J -> [wtile_h0, wtile_h1]
            qtn = [big.tile([P, QW], bf16, name=f"qt{i}") for i in range(4)]

            def emit_build_chunk(J, ci, on_act=False, use_zpool=False):
                qb = J * P
                c0, cw = ((0, QW), (QW, QW), (2 * QW, WER - 2 * QW))[ci]
                if ci == 0:
                    staging[J] = [
                        cpool.tile([P, WER], bf16, name=f"wst{h}", tag="wst")
                        for h in range(2)
                    ]
                if use_zpool:
                    pair = zpool.tile([P, 2, QW], f32, name="wpz", tag="zs")
                    pts = [pair[:, 0, :], pair[:, 1, :]]
                else:
                    pts = [
                        ppool.tile([P, QW], f32, name=f"wps{h}", tag="ps")[:]
                        for h in range(2)
                    ]
                for h in range(2):
                    hs = slice(h * 64, h * 64 + 64)
                    nc.tensor.matmul(
                        pts[h][:, :cw],
                        qtn[J // 4][hs, (J % 4) * P : (J % 4 + 1) * P],
                        relx_sb[hs, c0 : c0 + cw],
                        start=True,
                        stop=True,
                    )
                for h in range(2):
                    if on_act and h == 0:
                        nc.scalar.copy(
                            staging[J][h][:, c0 : c0 + cw], pts[h][:, :cw]
                        )
                    else:
                        nc.vector.tensor_copy(
                            staging[J][h][:, c0 : c0 + cw], pts[h][:, :cw]
                        )

            def emit_shear(J):
                qb = J * P
                r0 = max(0, qb - 512)
                r1 = min(N, qb + 640)
                rw = r1 - r0
                for h in range(2):
                    wt = wpool.tile([P, 1152], bf16, name=f"win{h}", tag="win")
                    src = AP(
                        tensor=staging[J][h][:].tensor,
                        offset=640 + r0 - qb,
                        ap=[[WER - 1, P], [1, rw]],
                    )
                    nc.gpsimd.dma_start(wt[:, :rw], src)
                    windows[(h, J)] = (wt, r0)
                del staging[J]

            def build_filler_js(js, on_act=False, use_zpool=False):
                out = []
                for J in js:
                    for ci in range(3):
                        out.append(
                            lambda J=J, ci=ci: emit_build_chunk(
                                J, ci, on_act, use_zpool
                            )
                        )
                    out.append(lambda J=J: emit_shear(J))
                return out

            bitems0 = build_filler_js([3, 2, 1, 0], on_act=True)
            ebi = 0

            # ---- projections: qT2/kT2 ----
            kt2 = big.tile([P, N], bf16)
            for nch in range(4):
                for which, wsb in (("q", wq_sb), ("k", wk_sb)):
                    pt = ppool.tile([P, QW], f32, name="proj", tag="ps")
                    for cc in range(4):
                        nc.tensor.matmul(
                            pt[:],
                            wsb[:, cc, :],
                            xtn[nch][:, cc, :],
                            start=(cc == 0),
                            stop=(cc == 3),
                        )
                    if which == "q":
                        nc.vector.tensor_copy(qtn[nch][:], pt[:])
                    else:
                        nc.vector.tensor_copy(
                            kt2[:, nch * QW : (nch + 1) * QW], pt[:]
                        )
                    if nch >= 1:
                        for _ in range(2):
                            if ebi < 8:
                                bitems0[ebi]()
                                ebi += 1

            ktp = big.tile([P, N], bf16)
            ktf = big.tile([P, N], bf16)
            nc.vector.tensor_scalar_add(ktp[:], kt2[:], edge_sb[:, 0:1])
            nc.vector.tensor_scalar_add(ktf[:], kt2[:], edge_sb[:, 1:2])

            # ---- v2 = [v | 1]: vproj for kt>=2 happens as filler inside
            # qc0's flash (keeps PE dense); lead phase = J0-3 builds only ----
            v2 = big.tile([P, 2, NT, 65], bf16)

            def emit_vproj(kt):
                pt = ppool.tile([P, QW], f32, name="vproj", tag="ps")
                for cc in range(4):
                    nc.tensor.matmul(
                        pt[:, :P],
                        xtn[kt // 4][:, cc, (kt % 4) * P : (kt % 4 + 1) * P],
                        wv_sb[:, cc, :],
                        start=(cc == 0),
                        stop=(cc == 3),
                    )
                nc.vector.tensor_copy(
                    v2[:, 0:2, kt, 0:64],
                    AP(
                        tensor=pt[:].tensor,
                        offset=0,
                        ap=[[QW, P], [64, 2], [1, 64]],
                    ),
                )

            nc.vector.memset(v2[:], 1.0)
            emit_vproj(NT - 1)
            emit_vproj(NT - 2)

            # ---- normalization + output projection (for a finished qc) ----
            otn = big.tile([64, 2, QW], bf16)
            sel_sb = const.tile([2, 2, 64], bf16)
            nc.gpsimd.dma_start(sel_sb[:], sel_d.ap())

            def build_filler(qc, on_act=False, use_zpool=False):
                return build_filler_js(
                    range(qc * NJ, (qc + 1) * NJ), on_act, use_zpool
                )

            def emit_norm_head(qc, ost, rrow):
                # den rows -> [64,16] -> reciprocal -> bf16 row form
                # (DVE reciprocal costs ~8 cyc/free-elem, so shrink the
                # free dim via DMA reshape before it)
                den16 = npool.tile([64, 16], f32, name="den16", tag="nrm")
                for h in range(2):
                    dst = AP(
                        tensor=den16[:].tensor,
                        offset=h * 8,
                        ap=[[16, 64], [1, 8]],
                    )
                    nc.sync.dma_start(dst, ost[h][64:65, :])
                rden16 = npool.tile([64, 16], f32, name="rden16", tag="nrm2")
                nc.vector.reciprocal(rden16[:], den16[:])
                rden16b = npool.tile([64, 16], bf16, name="rden16b", tag="nrm3")
                nc.vector.tensor_copy(rden16b[:], rden16[:])
                for h in range(2):
                    src = AP(
                        tensor=rden16b[:].tensor,
                        offset=h * 8,
                        ap=[[16, 64], [1, 8]],
                    )
                    dst = AP(
                        tensor=rrow[:].tensor,
                        offset=h * QW,
                        ap=[[QW, 1], [1, QW]],
                    )
                    nc.sync.dma_start(dst, src)

            def norm_filler(qc, ost, rrow, final=False):
                # emitted during qc+1 (or inline at the very end for the
                # final qc): broadcast recip + normalize + project
                out = []

                def emit_rcb_mul(h):
                    rcb = ppool.tile([64, QW], f32, name="rcb", tag="ps")
                    nc.tensor.matmul(
                        rcb[:], sel_sb[:, h, :], rrow[:], start=True, stop=True
                    )
                    nc.vector.tensor_mul(
                        otn[:, h, :], ost[h][0:64, :], rcb[:]
                    )

                def emit_yproj(nt):
                    pt = ppool.tile([P, QW], f32, name="yproj", tag="ps")
                    for h in range(2):
                        nc.tensor.matmul(
                            pt[:],
                            otn[:, h, nt * P : (nt + 1) * P],
                            wo_sb[:, h, :],
                            start=(h == 0),
                            stop=(h == 1),
                        )
                    yt = ypool.tile([P, QW], bf16, name="y_sb")
                    if final:
                        nc.scalar.copy(yt[:], pt[:])
                    else:
                        nc.vector.tensor_copy(yt[:], pt[:])
                    nc.sync.dma_start(
                        y_d.ap()[(qc * NJ + nt) * P : (qc * NJ + nt + 1) * P, :],
                        yt[:],
                    )

                out.append(lambda: emit_rcb_mul(0))
                out.append(lambda: emit_rcb_mul(1))
                for nt in range(NJ):
                    out.append(lambda nt=nt: emit_yproj(nt))
                return out

            # ---- flash loop ----
            filler = []
            pending_norm = None
            for qc in range(NQC):
                if qc == 0:
                    # qc0 runs kt DESCENDING: kt 15..8 need no J0-3 windows,
                    # so those builds become fillers (2 builds + 1 vproj per
                    # iteration; J3 first since descending kt hits it first)
                    bitems = bitems0[ebi:]
                    bitems += build_filler_js([4, 5, 6, 7])
                    vitems = [
                        lambda kt=k: emit_vproj(kt)
                        for k in range(NT - 3, -1, -1)
                    ]
                    bi = vi = 0
                    while bi < len(bitems) or vi < len(vitems):
                        for _ in range(2):
                            if bi < len(bitems):
                                filler.append(bitems[bi])
                                bi += 1
                        if vi < len(vitems):
                            filler.append(vitems[vi])
                            vi += 1
                else:
                    bitems = (
                        build_filler(qc + 1) if qc + 1 < NQC else []
                    )
                    nitems = pending_norm or []
                    pending_norm = None
                    # a few builds first (norm DMA chain needs ~4us of
                    # latency cover), then interleave norm work among the
                    # remaining builds so the qc boundary never drains a
                    # long dependent chain with PE idle
                    filler += bitems[:4]
                    rest = bitems[4:]
                    i = j = 0
                    while i < len(nitems) or j < len(rest):
                        if i < len(nitems):
                            filler.append(nitems[i])
                            i += 1
                        if j < len(rest):
                            filler.append(rest[j])
                            j += 1


                oth = [
                    opool.tile([65, QW], f32, name=f"oth{h}", tag="ot")
                    for h in range(2)
                ]
                kts = list(range(NT - 1, -1, -1)) if qc == 0 else list(range(NT))
                prev = None  # (at tile, kt) pending attn@v
                for kt in kts:
                    kb = kt * P
                    cls = []
                    for j in range(NJ):
                        dlt = (qc * NJ + j) * P - kb
                        cls.append(
                            "p" if dlt >= 640 else ("f" if dlt <= -640 else "w")
                        )
                    runs = []
                    for j in range(NJ):
                        if runs and runs[-1][2] == cls[j]:
                            runs[-1][1] += P
                        else:
                            runs.append([j * P, P, cls[j]])
                    zt = zpool.tile([P, 2, QW], f32, name="zt", tag="zs")
                    for ri, (s, wd, c) in enumerate(runs):
                        kvar = {"p": ktp, "f": ktf, "w": kt2}[c]
                        for h in range(2):
                            hs = slice(h * 64, h * 64 + 64)
                            nc.tensor.matmul(
                                zt[:, h, s : s + wd],
                                kvar[hs, kb : kb + P],
                                qtn[qc][hs, s : s + wd],
                                start=(ri == 0),
                                stop=False,
                                skip_group_check=True,
                            )
                    for h in range(2):
                        for j in range(NJ):
                            if cls[j] != "w":
                                continue
                            wt, r0 = windows[(h, qc * NJ + j)]
                            nc.tensor.matmul(
                                zt[:, h, j * P : (j + 1) * P],
                                wt[:, kb - r0 : kb - r0 + P],
                                ident_sb,
                                start=False,
                                stop=False,
                                skip_group_check=True,
                            )
                    at = apool.tile([P, 2, QW], bf16, name="attnT")
                    nc.scalar.activation(
                        at[:], zt[:], mybir.ActivationFunctionType.Exp,
                        scale=SCALE,
                    )
                    if prev is not None:
                        pat, pkt = prev
                        for h in range(2):
                            nc.tensor.matmul(
                                oth[h][:],
                                v2[:, h, pkt, :],
                                pat[:, h, :],
                                start=(pkt == kts[0]),
                                stop=False,
                                skip_group_check=True,
                            )
                    prev = (at, kt)
                    for _ in range(3 if qc == 0 else 2):
                        if filler:
                            filler.pop(0)()
                # last attn@v
                pat, pkt = prev
                for h in range(2):
                    nc.tensor.matmul(
                        oth[h][:],
                        v2[:, h, pkt, :],
                        pat[:, h, :],
                        start=False,
                        stop=True,
                        skip_group_check=True,
                    )
                while filler:
                    filler.pop(0)()
                # stage out + denominators for this qc
                fin = qc == NQC - 1
                ost = []
                for h in range(2):
                    o = spool.tile([65, QW], f32, name=f"ost{h}", tag="ost")
                    if fin and h == 1:
                        nc.vector.tensor_copy(o[:], oth[h][:])
                    else:
                        nc.scalar.copy(o[:], oth[h][:])
                    ost.append(o)
                rrow = npool.tile([2, QW], bf16, name="rrow", tag="nrmb")
                emit_norm_head(qc, ost, rrow)
                pending_norm = norm_filler(qc, ost, rrow, final=fin)

            for f in pending_norm:
                f()

    nc.compile()
    return nc


def _host_prep(x, Wq, Wkv, Wo, rel_emb):
    import ml_dtypes

    bf = ml_dtypes.bfloat16
    ident = np.eye(P, dtype=np.float32).astype(bf)
    sel2 = np.zeros((2, 2, 64), np.float32)
    for i in range(2):
        sel2[i, i, :] = 1.0
    sel2 = np.ascontiguousarray(sel2.reshape(2, 2 * 64)).astype(bf)
    relX = rel_emb[np.clip(1152 - np.arange(WER), 0, 1024)].T
    relx2 = np.ascontiguousarray(np.concatenate([relX, relX], axis=0).astype(bf))
    edge = np.stack([rel_emb[1024], rel_emb[0]], axis=1)
    edge2 = np.ascontiguousarray(
        np.concatenate([edge, edge], axis=0).astype(np.float32)
    )
    Wkv_r = Wkv.reshape(DIM, 2, HEADS, D)

    def wlayout(w):
        # [512, 128] -> [128 part, 4 chunk, 128] contiguous
        return np.ascontiguousarray(
            w.reshape(4, P, P).transpose(1, 0, 2).astype(bf)
        )

    in_maps = []
    for core in range(8):
        b = core // 4
        h0 = 2 * (core % 4)
        in_maps.append(
            {
                "xT4": np.ascontiguousarray(
                    x[b].T.astype(bf).reshape(4, P, 4, QW).transpose(2, 1, 0, 3)
                ),
                "wq2": wlayout(Wq[:, h0 * D : (h0 + 2) * D]),
                "wk2": wlayout(Wkv_r[:, 0, h0 : h0 + 2].reshape(DIM, 2 * D)),
                "wv2": wlayout(Wkv_r[:, 1, h0 : h0 + 2].reshape(DIM, 2 * D)),
                "wo2": np.ascontiguousarray(
                    Wo[h0 * D : (h0 + 2) * D, :]
                    .reshape(2, D, DIM)
                    .transpose(1, 0, 2)
                    .astype(bf)
                ),
                "relx2": relx2,
                "edge2": edge2,
                "identw": ident,
                "sel2": sel2,
            }
        )
    return in_maps


def kernel(x, Wq, Wkv, Wo, bo, rel_emb, _want_trace=False):
    from concourse.bass_utils import run_bass_kernel_spmd

    x = np.asarray(x)
    if "nc" not in _cached:
        _cached["nc"] = _build_program()
    nc = _cached["nc"]
    in_maps = _host_prep(x, np.asarray(Wq), np.asarray(Wkv), np.asarray(Wo),
                         np.asarray(rel_emb))
    res = run_bass_kernel_spmd(
        nc, in_maps, core_ids=list(range(8)), trace=_want_trace
    )
    _cached["last_result"] = res
    y = np.zeros((2, N, DIM), np.float32)
    for core in range(8):
        y[core // 4] += res.results[core]["y"].astype(np.float32)
    y += np.asarray(bo).astype(np.float32)[None, None, :]
    return y
